# revision 9
# baseline (speedup 1.0000x reference)
"""GridAttention Trainium2 kernel (v2: ramp/ring/engine-rebalance).

Full inputs -> full output. Internally shards (batch, head-pair) across 8
NeuronCores: core c handles batch c//4 and heads (2*(c%4), 2*(c%4)+1).

Math notes (unchanged from v1):
 - Attention scores are computed TRANSPOSED: S^T[j, i] = k_j . q_i * scale
   + rowbias[i, j], laid out [k partitions, q free]. Softmax-exp is
   elementwise, the denominator a matmul reduction (ones column in V), and
   P^T is directly the moving operand of the AV matmul.
 - ROW bias rides inside the QK matmul (contraction augmented to
   K=128 = [qk 64 | onehot(rj) 64] against [q 64 | rowr 64]; matmul cost
   on TRN2 is N-columns only, so free).
 - COL bias applied multiplicatively after exp; exp(colbias)^T tiles are
   periodic (9 distinct pair-blocks), host-precomputed (ecol).
 - No max-subtraction in softmax (logits ~ N(0,1), shift-invariant).
 - Device emits per-head UNNORMALIZED projected output + denominator row;
   host computes sum_h out_h / d_h.

v2 changes (trace-driven):
 - Engine floors per core: PE matmul ~152us, scalar exp 144x~1.07us
   ~ 154us. v1 wasted ~27us on a serial input-DMA ramp (first exp at
   27.7us, cold-clock matmuls at 427ns) and ~7us on a tail that
   serialized the last 4 output DMAs on one ring.
 - Input DMAs now split across all three DMA queues (sync/SP HWDGE,
   scalar/Act HWDGE, gpsimd SWDGE), with xT chunks halved and the
   first chunk quartered so projection matmuls start as data lands.
   Per-ring BW ~72GB/s, parallel across rings.
 - onehot(row) block [64, S] is generated on device from the identity
   via a stride-0 broadcast copy (DVE), saving 786KB of ramp DMA.
 - First four groups interleave heads ((0,0,0),(0,1,0),(0,0,1),(0,1,1))
   so early scores need only k-chunk 0 -> +2 groups of DMA slack on
   every later xT chunk.
 - Scalar engine does exp ONLY (it paces the kernel): all PSUM
   evacuations -> DVE; 36 of 144 col-bias multiplies -> GpSimd (idle
   otherwise); no scalar-queue DMA issues between first and last exp.
 - Outputs: outa on sync ring, outb on gpsimd ring; final 4 units +
   den on scalar ring (free after last exp) to compress the drain.
"""

import numpy as np

EMBED = 512
NH = 8
HD = 64
GH, GW = 64, 48
B = 2
S = GH * GW  # 3072
N_CORES = 8
NQ = S // 512  # 6 q chunks of 512
NM = S // 128  # 24 k chunks of 128
NG = NM // 2   # 12 groups of 2 k-chunks per (n, h)
KC = 4         # 512 = 4 contraction chunks of 128

_CACHE = {}


def _build_program():
    import concourse.bass as bass
    import concourse.tile as tile
    import concourse.mybir as mybir
    from concourse import bacc
    from concourse.bass import ts, ds
    from concourse.masks import make_identity

    f32 = mybir.dt.float32
    f16 = mybir.dt.float16
    EXP = mybir.ActivationFunctionType.Exp

    nc = bacc.Bacc("TRN2", target_bir_lowering=False, debug=False,
                   num_devices=N_CORES)

    def inp(name, shape):
        return nc.dram_tensor(name, shape, f16, kind="ExternalInput").ap()

    # host-prepacked layouts (see _prep_core_inputs)
    xT_d = inp("xT", [128, NQ * 2048])        # [p, n*2048 + c*512 + col]
    wqkv_d = inp("wqkv", [128, 3 * 512])      # [p, (q|k|v)*512 + c*128 + col]
    rowr_d = [inp(f"rowr{h}", [64, S]) for h in range(2)]      # rowr_h only
    ecol_d = [inp(f"ecol{h}", [128, 6144]) for h in range(2)]  # blocks 0,1,2,0
    wout_d = inp("wout", [HD, 2 * EMBED])
    outa_d = nc.dram_tensor("outa", [S, EMBED], f16, kind="ExternalOutput").ap()
    outb_d = nc.dram_tensor("outb", [S, EMBED], f16, kind="ExternalOutput").ap()
    den_d = nc.dram_tensor("den", [2, S], f16, kind="ExternalOutput").ap()

    with tile.TileContext(nc) as tc:
        with (
            tc.tile_pool(name="const", bufs=1) as cpool,
            tc.tile_pool(name="vtwp", bufs=3) as vtwp,
            tc.tile_pool(name="ptp", bufs=4) as ptp,
            tc.tile_pool(name="ptmp", bufs=6) as ptmp,
            tc.tile_pool(name="osb", bufs=3) as opool,
            tc.tile_pool(name="ps", bufs=2, space="PSUM") as ps,
        ):
            # ---- resident SBUF tensors ----
            xT = [cpool.tile([128, 2048], f16, tag=f"xT{n}", name=f"xT{n}")
                  for n in range(NQ)]
            wqkv = cpool.tile([128, 3 * 512], f16)
            wout = cpool.tile([HD, 2 * EMBED], f16)
            # augLR[h]: cols [0:S) = augL (k | onehot-row), [S:2S) = augR
            # (q | rowr_h)
            augLR = [cpool.tile([128, 2 * S], f16, tag=f"augLR{h}",
                                name=f"augLR{h}") for h in range(2)]
            ecol = [cpool.tile([128, 6144], f16, tag=f"ecol{h}",
                               name=f"ecol{h}") for h in range(2)]
            vv = [cpool.tile([128, NM * 65], f16, tag=f"vv{h}", name=f"vv{h}")
                  for h in range(2)]
            outT = [cpool.tile([65, S], f16, tag=f"outT{h}", name=f"outT{h}")
                    for h in range(2)]
            ident = cpool.tile([128, 128], f16)

            # gpsimd: identity first (gates warmup + ohr-gen), then the
            # warmup rhs slice of vv; the rest of the ones columns are
            # memset after the SWDGE issues below so transfers start early
            make_identity(nc, ident[:, :])
            nc.gpsimd.memset(vv[0][:, 0:512], 1.0)

            # ---- input DMA rings ------------------------------------
            # sync (SP HWDGE): xT0 quarters c0/c2 + a-halves of xT1,2,3,5
            # + b-half of xT5; later all 24 outa units + den0.
            nc.sync.dma_start(out=xT[0][:, 0:512], in_=xT_d[:, ds(0, 512)])
            nc.sync.dma_start(out=xT[0][:, 1024:1536],
                              in_=xT_d[:, ds(1024, 512)])
            for n, half in ((1, 0), (2, 0), (3, 0), (5, 0), (5, 1)):
                o = half * 1024
                nc.sync.dma_start(out=xT[n][:, o:o + 1024],
                                  in_=xT_d[:, ds(n * 2048 + o, 1024)])

            # scalar (Act HWDGE): xT0 quarters c1/c3, rowr n0 slices,
            # ecol h0/h1 panel-0 halves, xT4a. All issued before exp(0);
            # the queue is idle until then. NOTHING more until after the
            # last exp.
            nc.scalar.dma_start(out=xT[0][:, 512:1024],
                                in_=xT_d[:, ds(512, 512)])
            nc.scalar.dma_start(out=xT[0][:, 1536:2048],
                                in_=xT_d[:, ds(1536, 512)])
            nc.scalar.dma_start(out=augLR[0][64:128, ds(S, 512)],
                                in_=rowr_d[0][:, 0:512])
            nc.scalar.dma_start(out=augLR[1][64:128, ds(S, 512)],
                                in_=rowr_d[1][:, 0:512])
            nc.scalar.dma_start(out=ecol[0][:, 0:1024],
                                in_=ecol_d[0][:, 0:1024])
            nc.scalar.dma_start(out=ecol[1][:, 0:1024],
                                in_=ecol_d[1][:, 0:1024])
            nc.scalar.dma_start(out=ecol[0][:, 1024:2048],
                                in_=ecol_d[0][:, 1024:2048])
            nc.scalar.dma_start(out=ecol[1][:, 1024:2048],
                                in_=ecol_d[1][:, 1024:2048])
            nc.scalar.dma_start(out=xT[4][:, 0:1024],
                                in_=xT_d[:, ds(4 * 2048, 1024)])

            # gpsimd (SWDGE): wqkv thirds (k first: gates kproj0), then
            # b-halves of xT1..4; the lazy constants go out from loop
            # slots below.
            nc.gpsimd.dma_start(out=wqkv[:, 512:1024],
                                in_=wqkv_d[:, 512:1024])     # wk
            nc.gpsimd.dma_start(out=wqkv[:, 0:512],
                                in_=wqkv_d[:, 0:512])        # wq
            nc.gpsimd.dma_start(out=wqkv[:, 1024:1536],
                                in_=wqkv_d[:, 1024:1536])    # wv
            for n in (1, 2, 3, 4):
                nc.gpsimd.dma_start(out=xT[n][:, 1024:2048],
                                    in_=xT_d[:, ds(n * 2048 + 1024, 1024)])
            nc.gpsimd.memset(vv[0][:, 512:NM * 65], 1.0)
            nc.gpsimd.memset(vv[1][:, :], 1.0)

            # ---- onehot(row) [64, S] generated on device ------------
            # ohr = kron(I64, ones(1,48)): identity columns repeated 48x
            # via a stride-0 inner AP. Split at col 1056 (=48*22) so the
            # first piece lands before scores(0).
            def emit_ohr(h, c0, c1, eng):
                src = ident[0:64, c0:c1]
                src = bass.AP(src.tensor, src.offset, src.ap + [[0, 48]])
                dst = augLR[h][64:128, 48 * c0:48 * c1]
                dst = bass.AP(dst.tensor, dst.offset,
                              [dst.ap[0], [48, c1 - c0], [1, 48]])
                eng.tensor_copy(dst, src)

            # ---- main-loop building blocks ----
            # group order: first four interleave heads on (n=0, g=0/1) so
            # early scores only need k-chunk 0 (xT0); then h-blocks as v1.
            groups = [(0, 0, 0), (0, 1, 0), (0, 0, 1), (0, 1, 1)]
            groups += [(0, 0, g) for g in range(2, NG)]
            groups += [(0, 1, g) for g in range(2, NG)]
            for n in range(1, NQ):
                groups += [(n, 0, g) for g in range(NG)]
                groups += [(n, 1, g) for g in range(NG)]
            NGRP = len(groups)
            assert NGRP == NQ * 2 * NG

            live = {}
            acc = {}

            def emit_scores(i):
                n, h, g = groups[i]
                st = ps.tile([128, 1024], f32, tag="st", name="st")
                for k in range(2):
                    m = 2 * g + k
                    nc.tensor.matmul(st[:, ts(k, 512)],
                                     augLR[h][:, ts(m, 128)],
                                     augLR[h][:, ds(S + n * 512, 512)],
                                     start=True, stop=True)
                live[("st", i)] = st

            def emit_expmul(i):
                n, h, g = groups[i]
                st = live.pop(("st", i))
                pt = ptp.tile([128, 1024], f16, tag="pt", name="pt")
                nc.scalar.activation(pt[:, :], st[:, :], EXP)
                ptm = ptmp.tile([128, 1024], f16, tag="ptm", name="ptm")
                esl = ecol[h][:, ds((n % 3) * 2048 + (2 * g % 3) * 512,
                                    1024)]
                # ~1/4 of the multiplies on the otherwise-idle GpSimd
                # (2.03us there vs 0.64us on DVE; AV lag of 3 groups
                # gives the slack), the rest on DVE.
                if i % 4 == 1:
                    nc.gpsimd.tensor_mul(ptm[:, :], pt[:, :], esl)
                else:
                    nc.vector.tensor_mul(ptm[:, :], pt[:, :], esl)
                live[("ptm", i)] = ptm

            def emit_av(i):
                n, h, g = groups[i]
                ptm = live.pop(("ptm", i))
                if g == 0:
                    acc[(n, h)] = ps.tile([65, 512], f32, tag="acc",
                                          name="acc")
                a = acc[(n, h)]
                for k in range(2):
                    m = 2 * g + k
                    nc.tensor.matmul(a[:, :], vv[h][:, ds(m * 65, 65)],
                                     ptm[:, ts(k, 512)],
                                     start=(m == 0), stop=(m == NM - 1))
                if g == NG - 1:
                    nc.vector.tensor_copy(outT[h][:, ts(n, 512)], a[:, :])
                    del acc[(n, h)]

            def emit_tail_unit(t, h, tag="fp", cast_scalar=False,
                               ring="default"):
                fp = ps.tile([128, 512], f32, tag=tag, name="fp")
                nc.tensor.matmul(fp[:, :], outT[h][0:64, ts(t, 128)],
                                 wout[:, ds(h * EMBED, EMBED)],
                                 start=True, stop=True)
                osb = opool.tile([128, 512], f16, tag="osb", name="osb")
                if cast_scalar:
                    nc.scalar.copy(osb[:, :], fp[:, :])
                else:
                    nc.vector.tensor_copy(osb[:, :], fp[:, :])
                out_d = outa_d if h == 0 else outb_d
                if ring == "default":
                    eng = nc.sync if h == 0 else nc.gpsimd
                else:
                    eng = ring
                eng.dma_start(out=out_d[ts(t, 128), :], in_=osb[:, :])

            def proj(dst_tag, w_ofs, n, tag):
                """4 accumulating matmuls: project x chunk n (2 heads)."""
                p = ps.tile([128, 512], f32, tag=tag, name=f"p{dst_tag}")
                for c in range(KC):
                    nc.tensor.matmul(p[:, :], wqkv[:, ds(w_ofs + c * 128, 128)],
                                     xT[n][:, ts(c, 512)],
                                     start=(c == 0), stop=(c == KC - 1))
                return p

            def emit_kevac(n, pk):
                nc.vector.tensor_copy(augLR[0][0:64, ts(n, 512)], pk[0:64, :])
                nc.vector.tensor_copy(augLR[1][0:64, ts(n, 512)],
                                      pk[64:128, :])

            def emit_qproj_copy(pq, n):
                for h in range(2):
                    nc.vector.tensor_copy(augLR[h][0:64, ds(S + n * 512, 512)],
                                          pq[64 * h:64 * h + 64, :])

            def emit_vproj(n):
                pv = proj("v", 1024, n, "fp")
                vtw = vtwp.tile([128, 512], f16, tag="vtw", name="vtw")
                nc.vector.tensor_copy(vtw[:, :], pv[:, :])
                for mm in range(4):
                    m = n * 4 + mm
                    ptr = ps.tile([128, 128], f16, tag="fp", name="ptr")
                    nc.tensor.transpose(ptr[:, :], vtw[:, ts(mm, 128)],
                                        ident[:, :])
                    nc.vector.tensor_copy(vv[0][:, ds(m * 65, 64)],
                                          ptr[:, 0:64])
                    nc.vector.tensor_copy(vv[1][:, ds(m * 65, 64)],
                                          ptr[:, 64:128])

            # ---- ramp ----------------------------------------------
            # PE warmup on resident data while input DMAs land: keeps
            # HAM at 2.4GHz into the first projections.
            warm = ps.tile([128, 512], f32, tag="st", name="warm")

            def emit_warm(k):
                for _ in range(k):
                    nc.tensor.matmul(warm[:, :], ident[:, :], vv[0][:, 0:512],
                                     start=True, stop=True)

            # ohr pieces that gate scores(0) (cols 0:1056, both heads)
            emit_ohr(0, 0, 22, nc.vector)
            emit_ohr(1, 0, 22, nc.vector)

            emit_warm(6)
            pk0 = proj("k", 512, 0, "fp")
            emit_warm(2)
            pq0 = proj("q", 0, 0, "fp")
            emit_kevac(0, pk0)
            emit_qproj_copy(pq0, 0)

            emit_scores(0)   # (0,0,0)
            emit_expmul(0)
            emit_vproj(0)    # vv chunks 0..3; AV(G0) lands at loop i=3
            emit_warm(3)
            emit_scores(1)   # (0,1,0)
            emit_expmul(1)
            pk1 = proj("k", 512, 1, "fp")
            emit_kevac(1, pk1)

            # ---- main loop (AV lagged 3 groups behind scores/exp) ----
            pq = {}
            for i in range(2, NGRP):
                n, h, g = groups[i]
                # remaining ohr pieces (cols 1056:3072): h0 needed from
                # group (0,0,4) (~G6), h1 from (0,1,2) (~G14). Before
                # this group's mul so the DVE order is ohr-then-mul.
                if i == 2:
                    emit_ohr(0, 22, 64, nc.vector)
                if i == 4:
                    emit_ohr(1, 22, 64, nc.vector)
                emit_scores(i)
                emit_expmul(i)
                if i >= 3:
                    emit_av(i - 3)
                # v-proj j lands just before AV of (0,0,2j) consumes it
                # (vproj(0) was emitted in the ramp)
                if i % 2 == 1 and 3 <= i <= 11:
                    emit_vproj(i // 2)
                # k-proj n' must be evac'd before scores (0,0,2(n'-1)+2)
                # at G=2(n'-1)+4: emit matmuls at G=2n'-2
                if i % 2 == 0 and 2 <= i <= 8:
                    nk = i // 2 + 1
                    pkn = proj("k", 512, nk, "fp")
                    emit_kevac(nk, pkn)
                # lazy-constant DMA issues from gpsimd queue slots
                if i == 1 + 2:  # rowr0 n1..n5
                    nc.gpsimd.dma_start(out=augLR[0][64:128, ds(S + 512, 2560)],
                                        in_=rowr_d[0][:, ds(512, 2560)])
                if i == 3 + 2:  # rowr1 n1..n5
                    nc.gpsimd.dma_start(out=augLR[1][64:128, ds(S + 512, 2560)],
                                        in_=rowr_d[1][:, ds(512, 2560)])
                if i == 5 + 2:  # wout (first tail unit ~G16)
                    nc.gpsimd.dma_start(out=wout[:, :], in_=wout_d[:, :])
                if i == 9 + 2:  # ecol0 panel 1 ((1,0) muls ~G27)
                    nc.gpsimd.dma_start(out=ecol[0][:, 2048:4096],
                                        in_=ecol_d[0][:, 2048:4096])
                if i == 13 + 2:  # ecol1 panel 1
                    nc.gpsimd.dma_start(out=ecol[1][:, 2048:4096],
                                        in_=ecol_d[1][:, 2048:4096])
                if i == 17 + 2:  # ecol0 panel 2
                    nc.gpsimd.dma_start(out=ecol[0][:, 4096:6144],
                                        in_=ecol_d[0][:, 4096:6144])
                if i == 21 + 2:  # ecol1 panel 2
                    nc.gpsimd.dma_start(out=ecol[1][:, 4096:6144],
                                        in_=ecol_d[1][:, 4096:6144])
                # q-proj n=1 during the (0,1) block (v1: i==15/17)
                if (n, h, g) == (0, 1, 3):
                    pq[1] = proj("q", 0, 1, "fp")
                if (n, h, g) == (0, 1, 5):
                    emit_qproj_copy(pq.pop(1), 1)
                # output-projection tail units spread across groups.
                # h==1 branch waits until g>=4: outT[0] chunk n is only
                # written at the AV of (n,0,11), emitted 3 groups after
                # it — with the head-interleaved start that is i=16 for
                # n=0, so g=2 (G14) would read outT[0] uninitialized.
                if h == 0 and n >= 1 and 2 <= g < 6:
                    emit_tail_unit(4 * (n - 1) + (g - 2), 1)
                elif h == 1 and 4 <= g < 8:
                    emit_tail_unit(4 * n + (g - 4), 0)
                if h == 0 and g == 6 and 2 <= n + 2 < NQ:
                    pq[n + 2] = proj("q", 0, n + 2, "fp")
                if h == 0 and g == 8 and 2 <= n + 2 < NQ:
                    emit_qproj_copy(pq.pop(n + 2), n + 2)
            for i in range(NGRP - 3, NGRP):
                emit_av(i)
            # denominator rows (row 64 of outT = sum_k P). den0 on sync
            # (outT[0] completes ~13us before the end), den1 on scalar
            # (free after the last exp).
            nc.sync.dma_start(out=den_d[0:1, :], in_=outT[0][64:65, :])
            nc.scalar.dma_start(out=den_d[1:2, :], in_=outT[1][64:65, :])
            # final-chunk tail: 4 units, casts split DVE/scalar, DMAs
            # spread over three rings so the drain doesn't serialize
            tail_rings = [nc.sync, nc.gpsimd, nc.scalar, nc.scalar]
            for tt in range(4):
                emit_tail_unit(4 * (NQ - 1) + tt, 1,
                               tag="fp" if tt % 2 == 0 else "st",
                               cast_scalar=(tt % 2 == 1),
                               ring=tail_rings[tt])

    nc.compile()
    return nc


def _get_nc():
    if "nc" not in _CACHE:
        _CACHE["nc"] = _build_program()
    return _CACHE["nc"]


def _prep_core_inputs(x, w_qkv, w_out, rel_row_tab, rel_col_tab):
    """Per-core input dicts (host-side shard + constant precompute)."""
    bf = np.float16
    x = np.asarray(x, np.float32)
    w_qkv = np.asarray(w_qkv, np.float32)
    w_out = np.asarray(w_out, np.float32)
    rel_row_tab = np.asarray(rel_row_tab, np.float32)
    rel_col_tab = np.asarray(rel_col_tab, np.float32)

    ri = np.arange(S) // GW           # grid row of flat index
    # rowr[h][t, i] = rel_row_tab[ri[i] - t + 63, h]; idx in [0,126]
    row_idx = ri[None, :] - np.arange(64)[:, None] + 63   # [64, S]

    # ecol pair-blocks: for group g (k-chunks m=2g, 2g+1) and q chunk n,
    # the exp(colbias^T) tile depends only on (g%3, n%3): 9 blocks of
    # [128, 1024]. Layout: [128, q3 * 2048 + bp * 512 + ii]
    jj = np.arange(128)
    ii = np.arange(512)
    ecol_idx = np.zeros((3, 4, 128, 512), np.int64)
    for q3 in range(3):
        for bp in range(4):
            cj = (32 * (bp % 3) + jj) % 48
            c_i = (32 * q3 + ii) % 48
            ecol_idx[q3, bp] = c_i[None, :] - cj[:, None] + 47
    ecol_idx = ecol_idx.transpose(2, 0, 1, 3).reshape(128, 6144)

    scale = HD ** -0.5
    in_maps = []
    for c in range(N_CORES):
        b = c // 4
        h0 = 2 * (c % 4)
        h1 = h0 + 1
        xT = np.ascontiguousarray(x[b].reshape(S, EMBED).T)   # [E, S]
        # device layout: [p, n*2048 + c*512 + col] = xT[c*128+p, n*512+col]
        xTn = xT.reshape(KC, 128, NQ, 512).transpose(1, 2, 0, 3)
        def wslice(base, h):
            return w_qkv[:, base + h * HD: base + (h + 1) * HD]
        def pack(base, mul=1.0):
            w = np.concatenate([wslice(base, h0), wslice(base, h1)],
                               axis=1) * mul                  # [512, 128]
            return w.reshape(KC, 128, 128).transpose(1, 0, 2).reshape(128, 512)
        wqkv = np.concatenate([pack(0, scale), pack(EMBED), pack(2 * EMBED)],
                              axis=1)

        in_maps.append({
            "xT": np.ascontiguousarray(xTn.reshape(128, NQ * 2048)).astype(bf),
            "wqkv": np.ascontiguousarray(wqkv).astype(bf),
            "rowr0": np.ascontiguousarray(
                rel_row_tab[row_idx, h0]).astype(bf),
            "rowr1": np.ascontiguousarray(
                rel_row_tab[row_idx, h1]).astype(bf),
            "ecol0": np.exp(rel_col_tab[ecol_idx, h0]).astype(bf),
            "ecol1": np.exp(rel_col_tab[ecol_idx, h1]).astype(bf),
            "wout": np.concatenate(
                [w_out[h0 * HD:(h0 + 1) * HD, :],
                 w_out[h1 * HD:(h1 + 1) * HD, :]], axis=1).astype(bf),
        })
    return in_maps


def _run(inputs, trace=False):
    from concourse.bass_utils import run_bass_kernel_spmd
    nc = _get_nc()
    in_maps = _prep_core_inputs(**inputs)
    res = run_bass_kernel_spmd(nc, in_maps, list(range(N_CORES)), trace=trace)
    acc = np.zeros((B, S, EMBED), np.float32)
    for c in range(N_CORES):
        r = res.results[c]
        den = np.asarray(r["den"], np.float32)          # [2, S]
        acc[c // 4] += np.asarray(r["outa"], np.float32) / den[0][:, None]
        acc[c // 4] += np.asarray(r["outb"], np.float32) / den[1][:, None]
    return acc.reshape(B, GH, GW, EMBED), res


def kernel(x, w_qkv, w_out, rel_row_tab, rel_col_tab):
    out, _ = _run(dict(x=x, w_qkv=w_qkv, w_out=w_out,
                       rel_row_tab=rel_row_tab, rel_col_tab=rel_col_tab))
    return out
